# revision 18
# baseline (speedup 1.0000x reference)
"""Trainium2 Bass kernel for additive-attention scores (mixed bf16/fp8).

Computes, for B=32, S=2048, H=1024:
    out1   = key @ W1^T                                  [B, H]
    out2   = value @ W2^T                                [B, S, H]
    scores = einsum('bsh,h->bs', tanh(out1[:,None]+out2), v)

Sharding: data-parallel over batch B across 8 NeuronCores (4 batches per
core); weights replicated.

Mixed precision: the H=1024 contraction of value @ W2^T is split into
HB*128 bf16 rows plus HF*256 fp8-e4m3 rows computed with DoubleRow
matmuls (2 fp8 MACs/cell/cycle), so a [128 s, 1024 o] chunk costs
2*(HB + HF) N=512 matmuls instead of 16.  W2/W1 are pre-scaled by 64 so
the fp8 weights avoid the e4m3 subnormal range; the tanh activation
divides the PSUM result back by 64 (exact power-of-2).  Measured rel_l2
vs the f32 reference: ~3e-3 at HF=0, ~1.3e-2 at HF=1, ~1.75e-2 at HF=2.

The host pre-packs (pure layout marshalling + dtype rounding): value is
pre-transposed into [p, k, s] slabs (bf16 rows) and pair-packed
[p, j, i, s] slabs (fp8 rows), weights into transposed slabs.

Per core, work is 64 chunks of [128 s, 1024 h] processed as 32 super
chunks (one PSUM allocation each, 2 bufs):
  - HWDGE loads per chunk: contiguous vb (bf16) + v8 (fp8) slabs
  - PSUM bias seeding with 64*out1[b]: sub 0 on ACT, sub 1 on DVE
    (ACT also does the tanh; seeding both subs on ACT made it critical)
  - PE: 2*(HB+HF) accumulating matmuls per chunk (start=False onto the
    seeded bias; has_written=1 from a one-time priming pass)
  - ACT: tanh(psum * 1/64) -> bf16; DVE: *v + reduce -> score column
  - per batch: scores leave via a strided SWDGE DMA; the final batch is
    PE-transposed and leaves via one contiguous DMA.

No host-side device warmup: driving the PEs hard before the kernel
pushes the package into the P0 power state and the whole kernel then
runs at 2.0 GHz instead of 2.4 (measured 316us -> 266us for bf16).
"""

import os
import sys

import numpy as np

for _p in ("/opt/trn_rl_repo",):
    if os.path.isdir(_p) and _p not in sys.path:
        sys.path.insert(0, _p)

B, S, H = 32, 2048, 1024
N_CORES = 8
BPC = B // N_CORES  # batches per core

_CACHE = {}


def _build(bpc, s, hb=4, hf=2, warmup_mms=100, prefetch=6,
           flush_defer=2, tail_slices=2, vt_bufs=8):
    """Build + compile the per-core Bass program (same program on all cores)."""
    from contextlib import ExitStack

    import concourse.bass as bass  # noqa: F401
    import concourse.tile as tile
    from concourse import bacc, masks, mybir

    f32 = mybir.dt.float32
    bf16 = mybir.dt.bfloat16
    f8 = mybir.dt.float8e4
    Tanh = mybir.ActivationFunctionType.Tanh
    mult = mybir.AluOpType.mult
    DR = mybir.MatmulPerfMode.DoubleRow

    HC = H // 128   # h-chunks of w1t (8)
    SC = s // 128   # s-chunks per batch (16)
    SB = SC // 2    # super-chunks per batch (8)
    assert 128 * hb + 256 * hf == H
    assert s % 256 == 0

    nc = bacc.Bacc("TRN2", target_bir_lowering=False, debug=False)

    # value slabs are loaded one SUPER-chunk (2 s-chunks) per DMA: fewer
    # HWDGE descriptor slots (~600ns each on the queue) during ramp-up
    vb_d = nc.declare_dram_parameter("vb", [bpc, SC // 2, 128, 2 * hb * 128], bf16,
                                     isOutput=False) if hb else None
    v8_d = nc.declare_dram_parameter("v8", [bpc, SC // 2, 128, 2 * hf * 256], f8,
                                     isOutput=False) if hf else None
    w2b_d = nc.declare_dram_parameter("w2b", [hb, 128, H], bf16,
                                      isOutput=False) if hb else None
    w28_d = nc.declare_dram_parameter("w28", [hf, 128, 2 * H], f8,
                                      isOutput=False) if hf else None
    w1t_d = nc.declare_dram_parameter("w1t", [HC, 128, H], bf16, isOutput=False)
    keyt_d = nc.declare_dram_parameter("keyt", [HC, 128, bpc], bf16, isOutput=False)
    v128_d = nc.declare_dram_parameter("v128", [128, H], bf16, isOutput=False)
    eb_d = nc.declare_dram_parameter("eb", [bpc, bpc * 128], bf16, isOutput=False)
    # scores leave in [b, p, c] layout (score[b, 128c+p] = scores_pc[b, p, c]);
    # the host un-permutes.  One contiguous 8KB HWDGE DMA per batch instead of
    # a ~11us strided SWDGE flush (or a PE transpose) per batch.
    out_d = nc.declare_dram_parameter("scores_pc", [bpc, 128, SC], f32, isOutput=True)

    with tile.TileContext(nc) as tc, ExitStack() as ctx:
        const_pool = ctx.enter_context(tc.tile_pool(name="const", bufs=1))
        wt_pool = ctx.enter_context(tc.tile_pool(name="wt", bufs=1))
        mmps_pool = ctx.enter_context(tc.tile_pool(name="mmps", bufs=2, space="PSUM"))
        vtb_pool = ctx.enter_context(tc.tile_pool(name="vtb", bufs=vt_bufs))
        vt8_pool = ctx.enter_context(tc.tile_pool(name="vt8", bufs=vt_bufs))
        to_pool = ctx.enter_context(tc.tile_pool(name="to", bufs=4))
        scr_pool = ctx.enter_context(tc.tile_pool(name="scr", bufs=2))
        sco_pool = ctx.enter_context(tc.tile_pool(name="sco", bufs=1))
        tmp_pool = ctx.enter_context(tc.tile_pool(name="tmp", bufs=2))

        # ---- setup DMAs, spread across HWDGE queues so they overlap ----
        # sync: keyt, eb, w2b, then the vb chunk stream
        # scalar: w1t (gates the out1 path), then the v8 chunk stream
        # vector: w28, v128
        keyt = const_pool.tile([128, HC * bpc], bf16, name="keyt", tag="keyt")
        nc.sync.dma_start(keyt[:].rearrange("p (k b) -> p k b", k=HC),
                          keyt_d[:, :, :].rearrange("k p b -> p k b"))
        eb = const_pool.tile([bpc, bpc * 128], bf16, name="eb", tag="eb")
        nc.sync.dma_start(eb[:], eb_d[:, :])
        # w1t split across both queues: halves the out1 critical path
        w1t = wt_pool.tile([128, HC * H], bf16, name="w1t", tag="w1t")
        hk = HC // 2
        w1v = w1t[:].rearrange("p (k o) -> p k o", k=HC)
        nc.scalar.dma_start(w1v[:, 0:hk, :],
                            w1t_d[0:hk, :, :].rearrange("k p o -> p k o"))
        nc.sync.dma_start(w1v[:, hk:HC, :],
                          w1t_d[hk:HC, :, :].rearrange("k p o -> p k o"))
        if hb:
            w2b = wt_pool.tile([128, hb * H], bf16, name="w2b", tag="w2b")
            nc.sync.dma_start(w2b[:].rearrange("p (k o) -> p k o", k=hb),
                              w2b_d[:, :, :].rearrange("k p o -> p k o"))
        if hf:
            w28 = wt_pool.tile([128, hf * 2 * H], f8, name="w28", tag="w28")
            nc.scalar.dma_start(w28[:].rearrange("p (j x) -> p j x", j=hf),
                                w28_d[:, :, :].rearrange("j p x -> p j x"))
        v_bc = const_pool.tile([128, H], bf16, name="v_bc", tag="vbc")
        nc.sync.dma_start(v_bc[:], v128_d[:, :])

        # ---- constants ----
        ident = const_pool.tile([128, 128], f32, name="ident", tag="ident")
        masks.make_identity(nc, ident[:])
        identr = const_pool.tile([128, 128], bf16, name="identr", tag="identr")
        nc.vector.tensor_copy(identr[:], ident[:])
        dum = const_pool.tile([128, 512], bf16, name="dum", tag="dum")
        nc.gpsimd.memset(dum[:], 0.0)

        chunks = [(b, c) for b in range(bpc) for c in range(SC)]
        n = len(chunks)
        nsup = n // 2

        def emit_load(si):
            # loads BOTH chunks of super-chunk si; sync queue, never scalar:
            # a DMA descriptor waiting on a buffer-reuse semaphore would
            # head-of-line-block the ACT seed/tanh stream behind it
            # (measured 6.8us PE stalls + HAM re-throttle)
            b, sc = si // (SB), si % SB
            vtb = vt8 = None
            if hb:
                vtb = vtb_pool.tile([128, 2 * hb * 128], bf16, name="vtb", tag="vtb")
                nc.sync.dma_start(vtb[:], vb_d[b, sc, :, :])
            if hf:
                vt8 = vt8_pool.tile([128, 2 * hf * 256], f8, name="vt8", tag="vt8")
                nc.sync.dma_start(vt8[:], v8_d[b, sc, :, :])
            return vtb, vt8

        # ---- setup-phase PE work, all inside mmps buffer A (re-primed after)
        tA = mmps_pool.tile([128, 2 * H], f32, name="mmps_t", tag="mmps")

        # warmup: flip the PE HAM clock-gate to full rate during the DMA wait
        for _ in range(warmup_mms):
            nc.tensor.matmul(tA[0:128, 0:128], identr[:], identr[:],
                             start=True, stop=True)

        # out1 = key @ (64*W1)^T -> [bpc, H] bf16
        out1_sb = const_pool.tile([bpc, H], bf16, name="out1_sb", tag="out1")
        for half in range(2):
            reg = tA[0:bpc, half * 512 : half * 512 + 512]
            for k in range(HC):
                nc.tensor.matmul(
                    reg,
                    keyt[:, k * bpc : (k + 1) * bpc],
                    w1t[:, k * H + half * 512 : k * H + half * 512 + 512],
                    start=(k == 0),
                    stop=(k == HC - 1),
                )
            nc.vector.tensor_copy(out1_sb[:, half * 512 : half * 512 + 512], reg)

        # broadcast out1[b] across partitions via eb matmuls; copies alternate
        # ACT/DVE and the psum region rotates over tA's four banks.  One tile
        # PER BATCH so chunk 0's bias seed doesn't wait for batches 1-3.
        out1_bc = [
            const_pool.tile([128, H], f32, name=f"out1_bc{b}", tag=f"out1bc{b}")
            for b in range(bpc)
        ]
        for j in range(2 * bpc):
            b, half = j // 2, j % 2
            reg = tA[:, (j % 4) * 512 : (j % 4) * 512 + 512]
            nc.tensor.matmul(
                reg,
                eb[0:bpc, b * 128 : (b + 1) * 128],
                out1_sb[0:bpc, half * 512 : half * 512 + 512],
                start=True,
                stop=True,
            )
            dst = out1_bc[b][:, half * 512 : half * 512 + 512]
            if j % 2 == 0:
                nc.scalar.copy(dst, reg)
            else:
                nc.vector.tensor_copy(dst, reg)

        # prime both PSUM buffers: start/stop matmuls covering every element
        # set has_written=1, so all chunk matmuls run start=False and
        # accumulate onto the seeded out1 bias
        tB = mmps_pool.tile([128, 2 * H], f32, name="mmps_t", tag="mmps")
        for t in (tA, tB):
            for q in range(4):
                nc.tensor.matmul(t[:, q * 512 : q * 512 + 512], identr[:], dum[:],
                                 start=True, stop=True)

        # ---- per-batch score accumulators [128, SC] ----
        sc_acc = [
            sco_pool.tile([128, SC], f32, name=f"sacc{b}", tag=f"sacc{b}")
            for b in range(bpc)
        ]

        def emit_bias(si):
            b = chunks[2 * si][0]
            mm = mmps_pool.tile([128, 2 * H], f32, name="mmps_t", tag="mmps")
            # sub 0 seeded by ACT, sub 1 by DVE: neither engine is saturated
            nc.scalar.copy(mm[:, 0:H], out1_bc[b][:])
            nc.vector.tensor_copy(mm[:, H : 2 * H], out1_bc[b][:])
            return mm

        def emit_mm(si, mm, vts):
            vtb, vt8 = vts
            if hf:
                v8v = vt8[:].rearrange("p (u j i c) -> p u j i c", u=2, j=hf, i=2)
                w8v = w28[:].rearrange("p (j i o) -> p j i o", j=hf, i=2)
            for sub in range(2):
                base = sub * H
                vb_base = sub * hb * 128
                for k in range(hb):
                    for half in range(2):
                        nc.tensor.matmul(
                            mm[:, base + half * 512 : base + half * 512 + 512],
                            vtb[:, vb_base + k * 128 : vb_base + (k + 1) * 128],
                            w2b[:, k * H + half * 512 : k * H + half * 512 + 512],
                            start=False,
                            stop=False,
                            skip_group_check=True,
                        )
                if hf:
                    for j in range(hf):
                        for half in range(2):
                            nc.tensor.matmul(
                                mm[:, base + half * 512 : base + half * 512 + 512],
                                v8v[:, sub, j, :, :],
                                w8v[:, j, :, half * 512 : half * 512 + 512],
                                start=False,
                                stop=False,
                                skip_group_check=True,
                                perf_mode=DR,
                            )
            return mm

        def emit_post(si, mm, last=False):
            b, c0 = chunks[2 * si]
            for sub in range(2):
                c = c0 + sub
                if last and sub == 1 and tail_slices > 1:
                    ns_, w = tail_slices, H // tail_slices
                    tmp = []
                    for q in range(ns_):
                        sl = slice(sub * H + q * w, sub * H + q * w + w)
                        to = to_pool.tile([128, w], bf16, name="tos", tag="tos", bufs=2)
                        nc.scalar.activation(to[:], mm[:, sl], Tanh, scale=1.0 / 64)
                        scr = scr_pool.tile([128, w], bf16, name="scrs", tag="scrs", bufs=2)
                        t = tmp_pool.tile([128, 1], f32, name="tacc", tag=f"tacc{q}", bufs=1)
                        tmp.append(t)
                        nc.vector.scalar_tensor_tensor(
                            out=scr[:], in0=to[:], scalar=1.0,
                            in1=v_bc[:, q * w : q * w + w], op0=mult, op1=mult,
                            accum_out=t[:],
                        )
                    nc.vector.tensor_add(sc_acc[b][:, c : c + 1], tmp[0][:], tmp[1][:])
                    for q in range(2, ns_):
                        nc.vector.tensor_add(sc_acc[b][:, c : c + 1],
                                             sc_acc[b][:, c : c + 1], tmp[q][:])
                else:
                    to = to_pool.tile([128, H], bf16, name="to", tag="to")
                    nc.scalar.activation(to[:], mm[:, sub * H : sub * H + H], Tanh,
                                         scale=1.0 / 64)
                    scr = scr_pool.tile([128, H], bf16, name="scr", tag="scr")
                    nc.vector.scalar_tensor_tensor(
                        out=scr[:],
                        in0=to[:],
                        scalar=1.0,
                        in1=v_bc[:],
                        op0=mult,
                        op1=mult,
                        accum_out=sc_acc[b][:, c : c + 1],
                    )

        def emit_flush(b):
            # one contiguous 8KB HWDGE store per batch (64B runs/partition)
            nc.sync.dma_start(out_d[b, :, :], sc_acc[b][:])

        # ---- main pipeline over super-chunks ----
        vts = {}
        mms = {}
        for i in range(prefetch):
            vts[i] = emit_load(i)
        for si in range(nsup):
            if si + prefetch < nsup:
                vts[si + prefetch] = emit_load(si + prefetch)
            mm = emit_bias(si)
            mms[si] = emit_mm(si, mm, vts.pop(si))
            if si >= 1:
                emit_post(si - 1, mms[si - 1])
                if si < nsup - 1:
                    mms.pop(si - 1)
            sf = si - flush_defer
            if sf >= 0 and (sf + 1) % SB == 0 and sf // SB < bpc - 1:
                emit_flush(sf // SB)
        emit_post(nsup - 1, mms.pop(nsup - 1), last=True)
        mms.pop(nsup - 2)
        emit_flush(bpc - 1)

    nc.compile()
    return nc


def _get_nc(bpc=BPC, s=S, **kw):
    key = (bpc, s, tuple(sorted(kw.items())))
    if key not in _CACHE:
        _CACHE[key] = _build(bpc, s, **kw)
    return _CACHE[key]


def _prepack(key, value, W1, W2, v, bpc=BPC, n_cores=N_CORES, hb=4, hf=2):
    """Host-side layout marshalling: shard value/key over cores, pre-transpose
    and cast the replicated weights into the slab layouts the PE consumes."""
    import ml_dtypes

    bf16 = ml_dtypes.bfloat16
    e4 = ml_dtypes.float8_e4m3
    HC = H // 128
    SC = S // 128
    hsplit = hb * 128
    key = np.asarray(key, dtype=np.float32)
    value = np.asarray(value, dtype=np.float32)
    W1 = np.asarray(W1, dtype=np.float32)
    W2 = np.asarray(W2, dtype=np.float32)
    v = np.asarray(v, dtype=np.float32).reshape(-1)

    # [H, H] natural [o, h] -> transposed slabs; weights pre-scaled by 64
    # (fp8 rows must clear the e4m3 subnormal range; tanh divides back)
    w1t = np.ascontiguousarray(64.0 * W1.T).astype(bf16).reshape(HC, 128, H)
    w2t64 = 64.0 * W2.T  # [h, o]
    w2b = np.ascontiguousarray(w2t64[:hsplit]).astype(bf16).reshape(hb, 128, H) \
        if hb else None
    if hf:
        # w28[j, p, i, o] = 64*W2[o, hsplit + 256j + 128i + p]
        w28 = np.ascontiguousarray(
            w2t64[hsplit:].reshape(hf, 2, 128, H).transpose(0, 2, 1, 3)
        ).astype(e4).reshape(hf, 128, 2 * H)
    v128 = np.ascontiguousarray(np.broadcast_to(v[None, :], (128, H))).astype(bf16)
    eb = np.zeros((bpc, bpc * 128), dtype=bf16)
    for b in range(bpc):
        eb[b, b * 128 : (b + 1) * 128] = 1.0

    maps = []
    for i in range(n_cores):
        vi = value[i * bpc : (i + 1) * bpc]  # [bpc, S, H]
        m = {
            "w1t": w1t,
            "keyt": np.ascontiguousarray(
                key[i * bpc : (i + 1) * bpc].T.astype(bf16).reshape(HC, 128, bpc)
            ),
            "v128": v128,
            "eb": eb,
        }
        if hb:
            m["w2b"] = w2b
            # vb[b, sc, p, u, k, s] = value[b, 256sc+128u+s, 128k+p]
            vb = vi[:, :, :hsplit].reshape(bpc, SC // 2, 2, 128, hb, 128)
            m["vb"] = np.ascontiguousarray(
                vb.transpose(0, 1, 5, 2, 4, 3)
            ).astype(bf16).reshape(bpc, SC // 2, 128, 2 * hb * 128)
        if hf:
            m["w28"] = w28
            # v8[b, sc, p, u, j, i, s] = value[b, 256sc+128u+s, hsplit+256j+128i+p]
            v8 = vi[:, :, hsplit:].reshape(bpc, SC // 2, 2, 128, hf, 2, 128)
            m["v8"] = np.ascontiguousarray(
                v8.transpose(0, 1, 6, 2, 4, 5, 3)
            ).astype(e4).reshape(bpc, SC // 2, 128, 2 * hf * 256)
        maps.append(m)
    return maps


def run(key, value, W1, W2, v, trace=False, **build_kw):
    """Run on 8 NeuronCores; returns (scores [B, S], BassKernelResults)."""
    from concourse.bass_utils import run_bass_kernel_spmd

    nc = _get_nc(**build_kw)
    in_maps = _prepack(key, value, W1, W2, v,
                       hb=build_kw.get("hb", 4), hf=build_kw.get("hf", 2))
    res = run_bass_kernel_spmd(nc, in_maps, list(range(N_CORES)), trace=trace)
    # scores_pc[b, p, c] = score[b, 128c+p] -> un-permute on host
    scores = np.concatenate(
        [
            res.results[i]["scores_pc"].transpose(0, 2, 1).reshape(BPC, S)
            for i in range(N_CORES)
        ],
        axis=0,
    )
    return scores, res


def kernel(key, value, W1, W2, v):
    # Tracing needs an NTFF hook this image may lack; never trace when grading.
    os.environ.setdefault("BASS_NEVER_TRACE", "1")
    scores, _ = run(key, value, W1, W2, v)
    return scores.astype(np.float32)


# revision 23
# speedup vs baseline: 1.0511x; 1.0511x over previous
"""Trainium2 Bass kernel for additive-attention scores (mixed bf16/fp8).

Computes, for B=32, S=2048, H=1024:
    out1   = key @ W1^T                                  [B, H]
    out2   = value @ W2^T                                [B, S, H]
    scores = einsum('bsh,h->bs', tanh(out1[:,None]+out2), v)

Sharding: data-parallel over batch B across 8 NeuronCores (4 batches per
core); weights replicated.

Mixed precision: the H=1024 contraction of value @ W2^T is split into
HB*128 bf16 rows plus HF*256 fp8-e4m3 rows computed with DoubleRow
matmuls (2 fp8 MACs/cell/cycle), so a [128 s, 1024 o] chunk costs
2*(HB + HF) N=512 matmuls instead of 16.  W2/W1 are pre-scaled by 64 so
the fp8 weights avoid the e4m3 subnormal range; the tanh activation
divides the PSUM result back by 64 (exact power-of-2).  Measured rel_l2
vs the f32 reference: ~3e-3 at HF=0, ~1.3e-2 at HF=1, ~1.75e-2 at HF=2.

The host pre-packs (pure layout marshalling + dtype rounding): value is
pre-transposed into [p, k, s] slabs (bf16 rows) and pair-packed
[p, j, i, s] slabs (fp8 rows), weights into transposed slabs.

Per core, work is 64 chunks of [128 s, 1024 h] processed as 32 super
chunks (one PSUM allocation each, 2 bufs):
  - HWDGE loads per chunk: contiguous vb (bf16) + v8 (fp8) slabs
  - PSUM bias seeding with 64*out1[b]: sub 0 on ACT, sub 1 on DVE
    (ACT also does the tanh; seeding both subs on ACT made it critical)
  - PE: 2*(HB+HF) accumulating matmuls per chunk (start=False onto the
    seeded bias; has_written=1 from a one-time priming pass)
  - ACT: tanh(psum * 1/64) -> bf16; DVE: *v + reduce -> score column
  - per batch: scores leave via a strided SWDGE DMA; the final batch is
    PE-transposed and leaves via one contiguous DMA.

No host-side device warmup: driving the PEs hard before the kernel
pushes the package into the P0 power state and the whole kernel then
runs at 2.0 GHz instead of 2.4 (measured 316us -> 266us for bf16).
"""

import os
import sys

import numpy as np

for _p in ("/opt/trn_rl_repo",):
    if os.path.isdir(_p) and _p not in sys.path:
        sys.path.insert(0, _p)

B, S, H = 32, 2048, 1024
N_CORES = 8
BPC = B // N_CORES  # batches per core

_CACHE = {}


def _build(bpc, s, hb=4, hf=2, warmup_mms=100, prefetch=6,
           flush_defer=2, tail_slices=2, vt_bufs=8):
    """Build + compile the per-core Bass program (same program on all cores)."""
    from contextlib import ExitStack

    import concourse.bass as bass  # noqa: F401
    import concourse.tile as tile
    from concourse import bacc, masks, mybir

    f32 = mybir.dt.float32
    bf16 = mybir.dt.bfloat16
    f8 = mybir.dt.float8e4
    Tanh = mybir.ActivationFunctionType.Tanh
    mult = mybir.AluOpType.mult
    DR = mybir.MatmulPerfMode.DoubleRow

    HC = H // 128   # h-chunks of w1t (8)
    SC = s // 128   # s-chunks per batch (16)
    SB = SC // 2    # super-chunks per batch (8)
    assert 128 * hb + 256 * hf == H
    assert s % 256 == 0

    nc = bacc.Bacc("TRN2", target_bir_lowering=False, debug=False)

    # value slabs are loaded one SUPER-chunk (2 s-chunks) per DMA: fewer
    # HWDGE descriptor slots (~600ns each on the queue) during ramp-up
    vb_d = nc.declare_dram_parameter("vb", [bpc, SC // 2, 128, 2 * hb * 128], bf16,
                                     isOutput=False) if hb else None
    v8_d = nc.declare_dram_parameter("v8", [bpc, SC // 2, 128, 2 * hf * 256], f8,
                                     isOutput=False) if hf else None
    w2b_d = nc.declare_dram_parameter("w2b", [hb, 128, H], bf16,
                                      isOutput=False) if hb else None
    w28_d = nc.declare_dram_parameter("w28", [hf, 128, 2 * H], f8,
                                      isOutput=False) if hf else None
    w1t_d = nc.declare_dram_parameter("w1t", [HC, 128, H], bf16, isOutput=False)
    keyt_d = nc.declare_dram_parameter("keyt", [HC, 128, bpc], bf16, isOutput=False)
    v128_d = nc.declare_dram_parameter("v128", [128, H], bf16, isOutput=False)
    eb_d = nc.declare_dram_parameter("eb", [bpc, bpc * 128], bf16, isOutput=False)
    # scores leave in [b, p, c] layout (score[b, 128c+p] = scores_pc[b, p, c]);
    # the host un-permutes.  One contiguous 8KB HWDGE DMA per batch instead of
    # a ~11us strided SWDGE flush (or a PE transpose) per batch.
    out_d = nc.declare_dram_parameter("scores_pc", [bpc, 128, SC], f32, isOutput=True)

    with tile.TileContext(nc) as tc, ExitStack() as ctx:
        const_pool = ctx.enter_context(tc.tile_pool(name="const", bufs=1))
        wt_pool = ctx.enter_context(tc.tile_pool(name="wt", bufs=1))
        # one [128, H] psum tile PER CHUNK (not per super-chunk): Tile tracks
        # deps at tile granularity, so sub 0's tanh must not wait for sub 1's
        # matmuls.  4 tiles x 2 banks = all 8 PSUM banks.
        mmps_pool = ctx.enter_context(tc.tile_pool(name="mmps", bufs=4, space="PSUM"))
        vtb_pool = ctx.enter_context(tc.tile_pool(name="vtb", bufs=vt_bufs))
        vt8_pool = ctx.enter_context(tc.tile_pool(name="vt8", bufs=vt_bufs))
        to_pool = ctx.enter_context(tc.tile_pool(name="to", bufs=4))
        scr_pool = ctx.enter_context(tc.tile_pool(name="scr", bufs=2))
        sco_pool = ctx.enter_context(tc.tile_pool(name="sco", bufs=1))
        tmp_pool = ctx.enter_context(tc.tile_pool(name="tmp", bufs=2))

        # ---- setup DMAs, spread across HWDGE queues so they overlap ----
        # sync: keyt, eb, w2b, then the vb chunk stream
        # scalar: w1t (gates the out1 path), then the v8 chunk stream
        # vector: w28, v128
        keyt = const_pool.tile([128, HC * bpc], bf16, name="keyt", tag="keyt")
        nc.sync.dma_start(keyt[:].rearrange("p (k b) -> p k b", k=HC),
                          keyt_d[:, :, :].rearrange("k p b -> p k b"))
        eb = const_pool.tile([bpc, bpc * 128], bf16, name="eb", tag="eb")
        nc.sync.dma_start(eb[:], eb_d[:, :])
        # w1t split across both queues: halves the out1 critical path
        w1t = wt_pool.tile([128, HC * H], bf16, name="w1t", tag="w1t")
        hk = HC // 2
        w1v = w1t[:].rearrange("p (k o) -> p k o", k=HC)
        nc.scalar.dma_start(w1v[:, 0:hk, :],
                            w1t_d[0:hk, :, :].rearrange("k p o -> p k o"))
        nc.sync.dma_start(w1v[:, hk:HC, :],
                          w1t_d[hk:HC, :, :].rearrange("k p o -> p k o"))
        if hb:
            w2b = wt_pool.tile([128, hb * H], bf16, name="w2b", tag="w2b")
            nc.sync.dma_start(w2b[:].rearrange("p (k o) -> p k o", k=hb),
                              w2b_d[:, :, :].rearrange("k p o -> p k o"))
        if hf:
            w28 = wt_pool.tile([128, hf * 2 * H], f8, name="w28", tag="w28")
            nc.scalar.dma_start(w28[:].rearrange("p (j x) -> p j x", j=hf),
                                w28_d[:, :, :].rearrange("j p x -> p j x"))
        v_bc = const_pool.tile([128, H], bf16, name="v_bc", tag="vbc")
        nc.sync.dma_start(v_bc[:], v128_d[:, :])

        # ---- constants ----
        ident = const_pool.tile([128, 128], f32, name="ident", tag="ident")
        masks.make_identity(nc, ident[:])
        identr = const_pool.tile([128, 128], bf16, name="identr", tag="identr")
        nc.vector.tensor_copy(identr[:], ident[:])
        dum = const_pool.tile([128, 512], bf16, name="dum", tag="dum")
        nc.gpsimd.memset(dum[:], 0.0)

        chunks = [(b, c) for b in range(bpc) for c in range(SC)]
        n = len(chunks)
        nsup = n // 2

        def emit_load(si):
            # loads BOTH chunks of super-chunk si; sync queue, never scalar:
            # a DMA descriptor waiting on a buffer-reuse semaphore would
            # head-of-line-block the ACT seed/tanh stream behind it
            # (measured 6.8us PE stalls + HAM re-throttle)
            b, sc = si // (SB), si % SB
            vtb = vt8 = None
            if hb:
                vtb = vtb_pool.tile([128, 2 * hb * 128], bf16, name="vtb", tag="vtb")
                nc.sync.dma_start(vtb[:], vb_d[b, sc, :, :])
            if hf:
                vt8 = vt8_pool.tile([128, 2 * hf * 256], f8, name="vt8", tag="vt8")
                nc.sync.dma_start(vt8[:], v8_d[b, sc, :, :])
            return vtb, vt8

        # ---- setup-phase PE work, inside the 4 mmps pool tiles (re-primed
        # before the stream; priming start/stop matmuls set has_written=1 so
        # all chunk matmuls run start=False and accumulate onto the seeded
        # out1 bias)
        tA0 = mmps_pool.tile([128, H], f32, name="mmps_t", tag="mmps")
        tA1 = mmps_pool.tile([128, H], f32, name="mmps_t", tag="mmps")
        tB0 = mmps_pool.tile([128, H], f32, name="mmps_t", tag="mmps")
        tB1 = mmps_pool.tile([128, H], f32, name="mmps_t", tag="mmps")

        # warmup: flip the PE HAM clock-gate to full rate during the DMA wait
        for _ in range(warmup_mms):
            nc.tensor.matmul(tA0[0:128, 0:128], identr[:], identr[:],
                             start=True, stop=True)

        # out1 = key @ (64*W1)^T -> [bpc, H] bf16
        out1_sb = const_pool.tile([bpc, H], bf16, name="out1_sb", tag="out1")
        for half in range(2):
            reg = tA0[0:bpc, half * 512 : half * 512 + 512]
            for k in range(HC):
                nc.tensor.matmul(
                    reg,
                    keyt[:, k * bpc : (k + 1) * bpc],
                    w1t[:, k * H + half * 512 : k * H + half * 512 + 512],
                    start=(k == 0),
                    stop=(k == HC - 1),
                )
            nc.vector.tensor_copy(out1_sb[:, half * 512 : half * 512 + 512], reg)

        # broadcast out1[b] across partitions via eb matmuls; copies alternate
        # ACT/DVE and the psum region rotates over the 4 tA half-regions.  One
        # tile PER BATCH so chunk 0's bias seed doesn't wait for batches 1-3.
        # Priming/filler matmuls are interleaved so the PE never idles long
        # enough (~3.4us) for HAM to re-throttle while the copies drain.
        out1_bc = [
            const_pool.tile([128, H], f32, name=f"out1_bc{b}", tag=f"out1bc{b}")
            for b in range(bpc)
        ]

        def prime(t, reps=1):
            for _ in range(reps):
                for q in range(2):
                    nc.tensor.matmul(t[:, q * 512 : q * 512 + 512], identr[:],
                                     dum[:], start=True, stop=True)

        for j in range(2 * bpc):
            b, half = j // 2, j % 2
            treg = (tA0, tA1)[(j % 4) // 2]
            reg = treg[:, (j % 2) * 512 : (j % 2) * 512 + 512]
            nc.tensor.matmul(
                reg,
                eb[0:bpc, b * 128 : (b + 1) * 128],
                out1_sb[0:bpc, half * 512 : half * 512 + 512],
                start=True,
                stop=True,
            )
            dst = out1_bc[b][:, half * 512 : half * 512 + 512]
            if j % 2 == 0:
                nc.scalar.copy(dst, reg)
            else:
                nc.vector.tensor_copy(dst, reg)
            if j == 1:
                prime(tB0)
                prime(tB1)
            elif j in (3, 5):
                prime(tB0, reps=3)  # filler: keeps HAM warm during copies
                prime(tB1, reps=3)
        prime(tA0)
        prime(tA1)

        # ---- per-batch score accumulators [128, SC] ----
        sc_acc = [
            sco_pool.tile([128, SC], f32, name=f"sacc{b}", tag=f"sacc{b}")
            for b in range(bpc)
        ]

        def emit_bias(si):
            b = chunks[2 * si][0]
            mm0 = mmps_pool.tile([128, H], f32, name="mmps_t", tag="mmps")
            mm1 = mmps_pool.tile([128, H], f32, name="mmps_t", tag="mmps")
            # sub 0 seeded by ACT, sub 1 by DVE: neither engine is saturated
            nc.scalar.copy(mm0[:], out1_bc[b][:])
            nc.vector.tensor_copy(mm1[:], out1_bc[b][:])
            return mm0, mm1

        def emit_mm(si, mm, vts):
            vtb, vt8 = vts
            if hf:
                v8v = vt8[:].rearrange("p (u j i c) -> p u j i c", u=2, j=hf, i=2)
                w8v = w28[:].rearrange("p (j i o) -> p j i o", j=hf, i=2)
            for sub in range(2):
                vb_base = sub * hb * 128
                for k in range(hb):
                    for half in range(2):
                        nc.tensor.matmul(
                            mm[sub][:, half * 512 : half * 512 + 512],
                            vtb[:, vb_base + k * 128 : vb_base + (k + 1) * 128],
                            w2b[:, k * H + half * 512 : k * H + half * 512 + 512],
                            start=False,
                            stop=False,
                            skip_group_check=True,
                        )
                if hf:
                    for j in range(hf):
                        for half in range(2):
                            nc.tensor.matmul(
                                mm[sub][:, half * 512 : half * 512 + 512],
                                v8v[:, sub, j, :, :],
                                w8v[:, j, :, half * 512 : half * 512 + 512],
                                start=False,
                                stop=False,
                                skip_group_check=True,
                                perf_mode=DR,
                            )
            return mm

        def emit_post(si, mm, last=False):
            b, c0 = chunks[2 * si]
            for sub in range(2):
                c = c0 + sub
                if last and sub == 1 and tail_slices > 1:
                    ns_, w = tail_slices, H // tail_slices
                    tmp = []
                    for q in range(ns_):
                        sl = slice(q * w, q * w + w)
                        to = to_pool.tile([128, w], bf16, name="tos", tag="tos", bufs=2)
                        nc.scalar.activation(to[:], mm[sub][:, sl], Tanh, scale=1.0 / 64)
                        scr = scr_pool.tile([128, w], bf16, name="scrs", tag="scrs", bufs=2)
                        t = tmp_pool.tile([128, 1], f32, name="tacc", tag=f"tacc{q}", bufs=1)
                        tmp.append(t)
                        nc.vector.scalar_tensor_tensor(
                            out=scr[:], in0=to[:], scalar=1.0,
                            in1=v_bc[:, q * w : q * w + w], op0=mult, op1=mult,
                            accum_out=t[:],
                        )
                    nc.vector.tensor_add(sc_acc[b][:, c : c + 1], tmp[0][:], tmp[1][:])
                    for q in range(2, ns_):
                        nc.vector.tensor_add(sc_acc[b][:, c : c + 1],
                                             sc_acc[b][:, c : c + 1], tmp[q][:])
                else:
                    to = to_pool.tile([128, H], bf16, name="to", tag="to")
                    nc.scalar.activation(to[:], mm[sub][:], Tanh, scale=1.0 / 64)
                    scr = scr_pool.tile([128, H], bf16, name="scr", tag="scr")
                    nc.vector.scalar_tensor_tensor(
                        out=scr[:],
                        in0=to[:],
                        scalar=1.0,
                        in1=v_bc[:],
                        op0=mult,
                        op1=mult,
                        accum_out=sc_acc[b][:, c : c + 1],
                    )

        def emit_flush(b):
            # one contiguous 8KB HWDGE store per batch (64B runs/partition)
            nc.sync.dma_start(out_d[b, :, :], sc_acc[b][:])

        # ---- main pipeline over super-chunks ----
        vts = {}
        mms = {}
        for i in range(prefetch):
            vts[i] = emit_load(i)
        for si in range(nsup):
            if si + prefetch < nsup:
                vts[si + prefetch] = emit_load(si + prefetch)
            mm = emit_bias(si)
            mms[si] = emit_mm(si, mm, vts.pop(si))
            if si >= 1:
                emit_post(si - 1, mms[si - 1])
                if si < nsup - 1:
                    mms.pop(si - 1)
            sf = si - flush_defer
            if sf >= 0 and (sf + 1) % SB == 0 and sf // SB < bpc - 1:
                emit_flush(sf // SB)
        emit_post(nsup - 1, mms.pop(nsup - 1), last=True)
        mms.pop(nsup - 2)
        emit_flush(bpc - 1)

    nc.compile()
    return nc


def _get_nc(bpc=BPC, s=S, **kw):
    key = (bpc, s, tuple(sorted(kw.items())))
    if key not in _CACHE:
        _CACHE[key] = _build(bpc, s, **kw)
    return _CACHE[key]


def _prepack(key, value, W1, W2, v, bpc=BPC, n_cores=N_CORES, hb=4, hf=2):
    """Host-side layout marshalling: shard value/key over cores, pre-transpose
    and cast the replicated weights into the slab layouts the PE consumes."""
    import ml_dtypes

    bf16 = ml_dtypes.bfloat16
    e4 = ml_dtypes.float8_e4m3
    HC = H // 128
    SC = S // 128
    hsplit = hb * 128
    key = np.asarray(key, dtype=np.float32)
    value = np.asarray(value, dtype=np.float32)
    W1 = np.asarray(W1, dtype=np.float32)
    W2 = np.asarray(W2, dtype=np.float32)
    v = np.asarray(v, dtype=np.float32).reshape(-1)

    # [H, H] natural [o, h] -> transposed slabs; weights pre-scaled by 64
    # (fp8 rows must clear the e4m3 subnormal range; tanh divides back)
    w1t = np.ascontiguousarray(64.0 * W1.T).astype(bf16).reshape(HC, 128, H)
    w2t64 = 64.0 * W2.T  # [h, o]
    w2b = np.ascontiguousarray(w2t64[:hsplit]).astype(bf16).reshape(hb, 128, H) \
        if hb else None
    if hf:
        # w28[j, p, i, o] = 64*W2[o, hsplit + 256j + 128i + p]
        w28 = np.ascontiguousarray(
            w2t64[hsplit:].reshape(hf, 2, 128, H).transpose(0, 2, 1, 3)
        ).astype(e4).reshape(hf, 128, 2 * H)
    v128 = np.ascontiguousarray(np.broadcast_to(v[None, :], (128, H))).astype(bf16)
    eb = np.zeros((bpc, bpc * 128), dtype=bf16)
    for b in range(bpc):
        eb[b, b * 128 : (b + 1) * 128] = 1.0

    maps = []
    for i in range(n_cores):
        vi = value[i * bpc : (i + 1) * bpc]  # [bpc, S, H]
        m = {
            "w1t": w1t,
            "keyt": np.ascontiguousarray(
                key[i * bpc : (i + 1) * bpc].T.astype(bf16).reshape(HC, 128, bpc)
            ),
            "v128": v128,
            "eb": eb,
        }
        if hb:
            m["w2b"] = w2b
            # vb[b, sc, p, u, k, s] = value[b, 256sc+128u+s, 128k+p]
            vb = vi[:, :, :hsplit].reshape(bpc, SC // 2, 2, 128, hb, 128)
            m["vb"] = np.ascontiguousarray(
                vb.transpose(0, 1, 5, 2, 4, 3)
            ).astype(bf16).reshape(bpc, SC // 2, 128, 2 * hb * 128)
        if hf:
            m["w28"] = w28
            # v8[b, sc, p, u, j, i, s] = value[b, 256sc+128u+s, hsplit+256j+128i+p]
            v8 = vi[:, :, hsplit:].reshape(bpc, SC // 2, 2, 128, hf, 2, 128)
            m["v8"] = np.ascontiguousarray(
                v8.transpose(0, 1, 6, 2, 4, 5, 3)
            ).astype(e4).reshape(bpc, SC // 2, 128, 2 * hf * 256)
        maps.append(m)
    return maps


def run(key, value, W1, W2, v, trace=False, **build_kw):
    """Run on 8 NeuronCores; returns (scores [B, S], BassKernelResults)."""
    from concourse.bass_utils import run_bass_kernel_spmd

    nc = _get_nc(**build_kw)
    in_maps = _prepack(key, value, W1, W2, v,
                       hb=build_kw.get("hb", 4), hf=build_kw.get("hf", 2))
    res = run_bass_kernel_spmd(nc, in_maps, list(range(N_CORES)), trace=trace)
    # scores_pc[b, p, c] = score[b, 128c+p] -> un-permute on host
    scores = np.concatenate(
        [
            res.results[i]["scores_pc"].transpose(0, 2, 1).reshape(BPC, S)
            for i in range(N_CORES)
        ],
        axis=0,
    )
    return scores, res


def kernel(key, value, W1, W2, v):
    # Tracing needs an NTFF hook this image may lack; never trace when grading.
    os.environ.setdefault("BASS_NEVER_TRACE", "1")
    scores, _ = run(key, value, W1, W2, v)
    return scores.astype(np.float32)
